# revision 21
# baseline (speedup 1.0000x reference)
"""CBOW negative-sampling loss kernel for Trainium2 (8 NeuronCores, SPMD).

Reference computation (all fp32):
    vo_embed  = vo @ V                        # [128]
    vi_embed  = (U.T @ vi).mean(axis=1)       # [128]
    left      = log_sigmoid(vi_embed @ vo_embed)
    neg_embed = neg_samples @ U               # [20, 128]
    right     = sum(log_sigmoid(-(neg_embed @ vi_embed)))
    out       = -(left + right)

Strategy: shard the vocab dim (100000) across 8 cores (12500 rows each).
All the heavy work is 31 GEMVs sharing one contraction over vocab:
pack [vo | neg_0..neg_19 | vi_0..vi_9] (host, pure relayout) into a
31-column stationary operand; stream V|U row-chunks through the tensor
engine accumulating into PSUM.  Each core emits a [31, 256] partial; the
host sums partials over cores (the "psum"), averages the 10 vi rows, and
applies the scalar log-sigmoid epilogue.
"""

import numpy as np

import concourse.bacc as bacc
import concourse.bass as bass
import concourse.mybir as mybir
import concourse.tile as tile
from concourse.bass_utils import run_bass_kernel_spmd

# Problem shapes (hardcoded per spec nn_CBOW_55009941127479)
VOC = 100000
EMB = 128
CTX = 10
KNEG = 20
NCORES = 8
SHARD = VOC // NCORES          # 12500 vocab rows per core
KP = 125                       # contraction rows per matmul chunk (SBUF partitions)
NCHUNK = SHARD // KP           # 100 chunks per core
M = 1 + KNEG + CTX             # stationary columns: [vo, neg_0..19, vi_0..9]
# DMA slabs (chunks per slab). 8 chunks -> exactly 4096B per partition per
# transfer: SDMA engines pipeline 4KB packets at line rate (~26 GB/s each)
# but process larger descriptors at roughly half rate.
SLABS = [8] * 12 + [4]
USE_FP32R = True               # PE single-pass fp32r: 4x matmul throughput

# Vocab rows are processed in a p-major order within each slab so that every
# DMA is contiguous on both the DRAM and SBUF side:
#   shard row for (slab s, partition p, chunk-in-slab j) = s*KP*SLAB + p*SLAB + j
# The host packs lhsT in the same order, so all operands agree on the
# (equivalent, order-independent) contraction over vocab.

F32 = mybir.dt.float32
MM_DT = mybir.dt.float32r if USE_FP32R else F32


def build_nc():
    """Build the per-core Bass module (SPMD: same program on all 8 cores)."""
    nc = bacc.Bacc(
        "TRN2",
        target_bir_lowering=False,
        debug=False,
        num_devices=NCORES,
    )
    # DRAM inputs are declared float32r (bit-identical 4-byte layout; numpy
    # sees float32) so the DMAs are pure copies — the cast-during-DMA path
    # runs the SDMA engines at half rate.
    lhsT_d = nc.dram_tensor("lhsT_packed", [KP, NCHUNK * M], MM_DT,
                            kind="ExternalInput")
    V_d = nc.dram_tensor("V_s", [SHARD, EMB], MM_DT, kind="ExternalInput")
    U_d = nc.dram_tensor("U_s", [SHARD, EMB], MM_DT, kind="ExternalInput")
    out_d = nc.dram_tensor("partial", [M, 2 * EMB], F32, kind="ExternalOutput")

    with tile.TileContext(nc) as tc:
        with (
            tc.tile_pool(name="const", bufs=1) as cpool,
            tc.tile_pool(name="rhs", bufs=10) as rpool,
            tc.tile_pool(name="acc", bufs=1, space="PSUM") as ppool,
        ):
            lhsT_s = cpool.tile([KP, NCHUNK, M], MM_DT)
            lhsT3 = lhsT_d.rearrange("p (c m) -> p c m", m=M)

            # out[m, 0:128]   = w_m @ V_chunk   (used for m=0: vo)
            # out[m, 128:256] = w_m @ U_chunk   (used for m=1..30: neg, vi)
            acc = ppool.tile([M, 2 * EMB], F32)
            off = 0
            for L in SLABS:
                # All big transfers ride SWDGE (gpsimd), which sprays
                # descriptors across all 16 SDMA engines (the HWDGE dynamic
                # queues only fan out 5-wide). Every transfer is contiguous
                # in DRAM and per-partition contiguous in SBUF. lhsT streams
                # in per-slab pieces so the first matmul isn't gated on the
                # whole 1.5 MB. p-major row order within each slab:
                #   shard row = off*KP + p*L + j
                cs = slice(off, off + L)
                # lhsT rides the otherwise-idle ACT HWDGE ring; slab 0's
                # V/U ride the SP HWDGE ring (no Q7 library-load wait), so
                # useful bytes flow during the SWDGE warmup and the SWDGE
                # stream carries only the remaining V/U slabs.
                nc.scalar.dma_start(out=lhsT_s[:, cs, :], in_=lhsT3[:, cs, :])
                rhs = rpool.tile([KP, 2, L, EMB], MM_DT, tag="rhs")
                V3 = V_d[off * KP:(off + L) * KP, :].rearrange(
                    "(p j) e -> p j e", j=L)
                U3 = U_d[off * KP:(off + L) * KP, :].rearrange(
                    "(p j) e -> p j e", j=L)
                eng = nc.sync if off == 0 else nc.gpsimd
                eng.dma_start(out=rhs[:, 0, :, :], in_=V3)
                eng.dma_start(out=rhs[:, 1, :, :], in_=U3)
                for j in range(L):
                    c = off + j
                    nc.tensor.matmul(
                        out=acc[:, :],
                        lhsT=lhsT_s[:, c, :],
                        rhs=rhs[:, :, j, :],
                        start=(c == 0),
                        stop=(c == NCHUNK - 1),
                    )
                off += L

            out_s = cpool.tile([M, 2 * EMB], F32)
            nc.vector.tensor_copy(out_s[:, :], acc[:, :])
            nc.sync.dma_start(out=out_d[:, :], in_=out_s[:, :])
    nc.compile()
    return nc


def make_in_maps(vo, vi, neg_samples, V, U):
    """Shard + relayout the full inputs into 8 per-core input maps.

    Host work is pure data movement: slicing, stacking and axis
    permutation. No arithmetic on values happens here.
    """
    vo = np.asarray(vo, dtype=np.float32)
    vi = np.asarray(vi, dtype=np.float32)
    neg = np.asarray(neg_samples, dtype=np.float32)
    V = np.asarray(V, dtype=np.float32)
    U = np.asarray(U, dtype=np.float32)

    in_maps = []
    for r in range(NCORES):
        lo, hi = r * SHARD, (r + 1) * SHARD
        # [12500, 31] = [vo | neg.T | vi] for this vocab shard
        W = np.concatenate([vo[lo:hi, None], neg[:, lo:hi].T, vi[lo:hi]],
                           axis=1)
        # p-major order within each slab: shard row = off*KP + p*L + j
        pieces = []
        off = 0
        for L in SLABS:
            seg = W[off * KP:(off + L) * KP].reshape(KP, L, M)
            pieces.append(seg)
            off += L
        lhsT_packed = np.ascontiguousarray(
            np.concatenate(pieces, axis=1)).reshape(KP, NCHUNK * M)
        in_maps.append({
            "lhsT_packed": lhsT_packed,
            "V_s": np.ascontiguousarray(V[lo:hi]),
            "U_s": np.ascontiguousarray(U[lo:hi]),
        })
    return in_maps


def combine_partials(partials):
    """Sum per-core partials and apply the scalar epilogue."""
    P = np.zeros((M, 2 * EMB), dtype=np.float64)
    for p in partials:
        P += p.astype(np.float64)
    vo_embed = P[0, :EMB]
    neg_embed = P[1:1 + KNEG, EMB:]
    vi_embed = P[1 + KNEG:, EMB:].sum(axis=0) / CTX

    def log_sigmoid(x):
        return -np.logaddexp(0.0, -x)

    left = log_sigmoid(vi_embed @ vo_embed)
    right = np.sum(log_sigmoid(-(neg_embed @ vi_embed)))
    return np.float32(-(left + right))


_NC = None


def kernel(vo, vi, neg_samples, V, U):
    global _NC
    if _NC is None:
        _NC = build_nc()
    in_maps = make_in_maps(vo, vi, neg_samples, V, U)
    res = run_bass_kernel_spmd(_NC, in_maps, list(range(NCORES)))
    return combine_partials([res.results[r]["partial"] for r in range(NCORES)])


# revision 23
# speedup vs baseline: 1.0137x; 1.0137x over previous
"""CBOW negative-sampling loss kernel for Trainium2 (8 NeuronCores, SPMD).

Reference computation (all fp32):
    vo_embed  = vo @ V                        # [128]
    vi_embed  = (U.T @ vi).mean(axis=1)       # [128]
    left      = log_sigmoid(vi_embed @ vo_embed)
    neg_embed = neg_samples @ U               # [20, 128]
    right     = sum(log_sigmoid(-(neg_embed @ vi_embed)))
    out       = -(left + right)

Strategy: shard the vocab dim (100000) across 8 cores (12500 rows each).
All the heavy work is 31 GEMVs sharing one contraction over vocab:
pack [vo | neg_0..neg_19 | vi_0..vi_9] (host, pure relayout) into a
31-column stationary operand; stream V|U row-chunks through the tensor
engine accumulating into PSUM.  Each core emits a [31, 256] partial; the
host sums partials over cores (the "psum"), averages the 10 vi rows, and
applies the scalar log-sigmoid epilogue.
"""

import numpy as np

import concourse.bacc as bacc
import concourse.bass as bass
import concourse.mybir as mybir
import concourse.tile as tile
from concourse.bass_utils import run_bass_kernel_spmd

# Problem shapes (hardcoded per spec nn_CBOW_55009941127479)
VOC = 100000
EMB = 128
CTX = 10
KNEG = 20
NCORES = 8
SHARD = VOC // NCORES          # 12500 vocab rows per core
KP = 125                       # contraction rows per matmul chunk (SBUF partitions)
NCHUNK = SHARD // KP           # 100 chunks per core
M = 1 + KNEG + CTX             # stationary columns: [vo, neg_0..19, vi_0..9]
# DMA slabs (chunks per slab). 8 chunks -> exactly 4096B per partition per
# transfer: SDMA engines pipeline 4KB packets at line rate (~26 GB/s each)
# but process larger descriptors at roughly half rate.
SLABS = [8] * 12 + [4]
USE_FP32R = True               # PE single-pass fp32r: 4x matmul throughput

# Vocab rows are processed in a p-major order within each slab so that every
# DMA is contiguous on both the DRAM and SBUF side:
#   shard row for (slab s, partition p, chunk-in-slab j) = s*KP*SLAB + p*SLAB + j
# The host packs lhsT in the same order, so all operands agree on the
# (equivalent, order-independent) contraction over vocab.

F32 = mybir.dt.float32
MM_DT = mybir.dt.float32r if USE_FP32R else F32


def build_nc():
    """Build the per-core Bass module (SPMD: same program on all 8 cores)."""
    nc = bacc.Bacc(
        "TRN2",
        target_bir_lowering=False,
        debug=False,
        num_devices=NCORES,
    )
    # DRAM inputs are declared float32r (bit-identical 4-byte layout; numpy
    # sees float32) so the DMAs are pure copies — the cast-during-DMA path
    # runs the SDMA engines at half rate.
    lhsT_d = nc.dram_tensor("lhsT_packed", [KP, NCHUNK * M], MM_DT,
                            kind="ExternalInput")
    V_d = nc.dram_tensor("V_s", [SHARD, EMB], MM_DT, kind="ExternalInput")
    U_d = nc.dram_tensor("U_s", [SHARD, EMB], MM_DT, kind="ExternalInput")
    out_d = nc.dram_tensor("partial", [M, 2 * EMB], F32, kind="ExternalOutput")

    with tile.TileContext(nc) as tc:
        with (
            tc.tile_pool(name="const", bufs=1) as cpool,
            tc.tile_pool(name="rhs", bufs=6) as rpool,
            tc.tile_pool(name="acc", bufs=1, space="PSUM") as ppool,
        ):
            lhsT_s = cpool.tile([KP, NCHUNK, M], MM_DT)
            lhsT3 = lhsT_d.rearrange("p (c m) -> p c m", m=M)

            # out[m, 0:128]   = w_m @ V_chunk   (used for m=0: vo)
            # out[m, 128:256] = w_m @ U_chunk   (used for m=1..30: neg, vi)
            acc = ppool.tile([M, 2 * EMB], F32)
            off = 0
            for L in SLABS:
                # All big transfers ride SWDGE (gpsimd), which sprays
                # descriptors across all 16 SDMA engines (the HWDGE dynamic
                # queues only fan out 5-wide). Every transfer is contiguous
                # in DRAM and per-partition contiguous in SBUF. lhsT streams
                # in per-slab pieces so the first matmul isn't gated on the
                # whole 1.5 MB. p-major row order within each slab:
                #   shard row = off*KP + p*L + j
                cs = slice(off, off + L)
                nc.gpsimd.dma_start(out=lhsT_s[:, cs, :], in_=lhsT3[:, cs, :])
                rhs = rpool.tile([KP, 2, L, EMB], MM_DT, tag="rhs")
                V3 = V_d[off * KP:(off + L) * KP, :].rearrange(
                    "(p j) e -> p j e", j=L)
                U3 = U_d[off * KP:(off + L) * KP, :].rearrange(
                    "(p j) e -> p j e", j=L)
                nc.gpsimd.dma_start(out=rhs[:, 0, :, :], in_=V3)
                nc.gpsimd.dma_start(out=rhs[:, 1, :, :], in_=U3)
                for j in range(L):
                    c = off + j
                    nc.tensor.matmul(
                        out=acc[:, :],
                        lhsT=lhsT_s[:, c, :],
                        rhs=rhs[:, :, j, :],
                        start=(c == 0),
                        stop=(c == NCHUNK - 1),
                    )
                off += L

            out_s = cpool.tile([M, 2 * EMB], F32)
            nc.vector.tensor_copy(out_s[:, :], acc[:, :])
            nc.sync.dma_start(out=out_d[:, :], in_=out_s[:, :])
    nc.compile()
    return nc


def make_in_maps(vo, vi, neg_samples, V, U):
    """Shard + relayout the full inputs into 8 per-core input maps.

    Host work is pure data movement: slicing, stacking and axis
    permutation. No arithmetic on values happens here.
    """
    vo = np.asarray(vo, dtype=np.float32)
    vi = np.asarray(vi, dtype=np.float32)
    neg = np.asarray(neg_samples, dtype=np.float32)
    V = np.asarray(V, dtype=np.float32)
    U = np.asarray(U, dtype=np.float32)

    in_maps = []
    for r in range(NCORES):
        lo, hi = r * SHARD, (r + 1) * SHARD
        # [12500, 31] = [vo | neg.T | vi] for this vocab shard
        W = np.concatenate([vo[lo:hi, None], neg[:, lo:hi].T, vi[lo:hi]],
                           axis=1)
        # p-major order within each slab: shard row = off*KP + p*L + j
        pieces = []
        off = 0
        for L in SLABS:
            seg = W[off * KP:(off + L) * KP].reshape(KP, L, M)
            pieces.append(seg)
            off += L
        lhsT_packed = np.ascontiguousarray(
            np.concatenate(pieces, axis=1)).reshape(KP, NCHUNK * M)
        in_maps.append({
            "lhsT_packed": lhsT_packed,
            "V_s": np.ascontiguousarray(V[lo:hi]),
            "U_s": np.ascontiguousarray(U[lo:hi]),
        })
    return in_maps


def combine_partials(partials):
    """Sum per-core partials and apply the scalar epilogue."""
    P = np.zeros((M, 2 * EMB), dtype=np.float64)
    for p in partials:
        P += p.astype(np.float64)
    vo_embed = P[0, :EMB]
    neg_embed = P[1:1 + KNEG, EMB:]
    vi_embed = P[1 + KNEG:, EMB:].sum(axis=0) / CTX

    def log_sigmoid(x):
        return -np.logaddexp(0.0, -x)

    left = log_sigmoid(vi_embed @ vo_embed)
    right = np.sum(log_sigmoid(-(neg_embed @ vi_embed)))
    return np.float32(-(left + right))


_NC = None


def kernel(vo, vi, neg_samples, V, U):
    global _NC
    if _NC is None:
        _NC = build_nc()
    in_maps = make_in_maps(vo, vi, neg_samples, V, U)
    res = run_bass_kernel_spmd(_NC, in_maps, list(range(NCORES)))
    return combine_partials([res.results[r]["partial"] for r in range(NCORES)])


# revision 25
# speedup vs baseline: 1.0287x; 1.0148x over previous
"""CBOW negative-sampling loss kernel for Trainium2 (8 NeuronCores, SPMD).

Reference computation (all fp32):
    vo_embed  = vo @ V                        # [128]
    vi_embed  = (U.T @ vi).mean(axis=1)       # [128]
    left      = log_sigmoid(vi_embed @ vo_embed)
    neg_embed = neg_samples @ U               # [20, 128]
    right     = sum(log_sigmoid(-(neg_embed @ vi_embed)))
    out       = -(left + right)

Strategy: shard the vocab dim (100000) across 8 cores (12500 rows each).
All the heavy work is 31 GEMVs sharing one contraction over vocab:
pack [vo | neg_0..neg_19 | vi_0..vi_9] (host, pure relayout) into a
31-column stationary operand; stream V|U row-chunks through the tensor
engine accumulating into PSUM.  Each core emits a [31, 256] partial; the
host sums partials over cores (the "psum"), averages the 10 vi rows, and
applies the scalar log-sigmoid epilogue.
"""

import numpy as np

import concourse.bacc as bacc
import concourse.bass as bass
import concourse.mybir as mybir
import concourse.tile as tile
from concourse.bass_utils import run_bass_kernel_spmd

# Problem shapes (hardcoded per spec nn_CBOW_55009941127479)
VOC = 100000
EMB = 128
CTX = 10
KNEG = 20
NCORES = 8
SHARD = VOC // NCORES          # 12500 vocab rows per core
KP = 125                       # contraction rows per matmul chunk (SBUF partitions)
NCHUNK = SHARD // KP           # 100 chunks per core
M = 1 + KNEG + CTX             # stationary columns: [vo, neg_0..19, vi_0..9]
# DMA slabs (chunks per slab). 8 chunks -> exactly 4096B per partition per
# transfer: SDMA engines pipeline 4KB packets at line rate (~26 GB/s each)
# but process larger descriptors at roughly half rate.
SLABS = [8] * 12 + [4]
USE_FP32R = True               # PE single-pass fp32r: 4x matmul throughput

# Vocab rows are processed in a p-major order within each slab so that every
# DMA is contiguous on both the DRAM and SBUF side:
#   shard row for (slab s, partition p, chunk-in-slab j) = s*KP*SLAB + p*SLAB + j
# The host packs lhsT in the same order, so all operands agree on the
# (equivalent, order-independent) contraction over vocab.

F32 = mybir.dt.float32
MM_DT = mybir.dt.float32r if USE_FP32R else F32


def build_nc():
    """Build the per-core Bass module (SPMD: same program on all 8 cores)."""
    nc = bacc.Bacc(
        "TRN2",
        target_bir_lowering=False,
        debug=False,
        num_devices=NCORES,
    )
    # DRAM inputs are declared float32r (bit-identical 4-byte layout; numpy
    # sees float32) so the DMAs are pure copies — the cast-during-DMA path
    # runs the SDMA engines at half rate.
    lhsT_d = nc.dram_tensor("lhsT_packed", [KP, NCHUNK * M], MM_DT,
                            kind="ExternalInput")
    V_d = nc.dram_tensor("V_s", [SHARD, EMB], MM_DT, kind="ExternalInput")
    U_d = nc.dram_tensor("U_s", [SHARD, EMB], MM_DT, kind="ExternalInput")
    out_d = nc.dram_tensor("partial", [M, 2 * EMB], F32, kind="ExternalOutput")

    with tile.TileContext(nc) as tc:
        with (
            tc.tile_pool(name="const", bufs=1) as cpool,
            tc.tile_pool(name="rhs", bufs=6) as rpool,
            tc.tile_pool(name="acc", bufs=1, space="PSUM") as ppool,
        ):
            lhsT_s = cpool.tile([KP, NCHUNK, M], MM_DT)
            lhsT3 = lhsT_d.rearrange("p (c m) -> p c m", m=M)

            # lhsT rides the ACT HWDGE ring, which is otherwise idle: its
            # bytes flow during the kernel entry window (before the SWDGE
            # Q7 library load finishes) instead of competing with the V/U
            # stream for the shared ~205 GB/s DMA budget. A small first
            # piece unblocks chunk 0's matmul early.
            nc.scalar.dma_start(out=lhsT_s[:, 0:16, :], in_=lhsT3[:, 0:16, :])
            nc.scalar.dma_start(out=lhsT_s[:, 16:, :], in_=lhsT3[:, 16:, :])

            # out[m, 0:128]   = w_m @ V_chunk   (used for m=0: vo)
            # out[m, 128:256] = w_m @ U_chunk   (used for m=1..30: neg, vi)
            acc = ppool.tile([M, 2 * EMB], F32)
            off = 0
            for L in SLABS:
                # All big transfers ride SWDGE (gpsimd), which sprays
                # descriptors across all 16 SDMA engines (the HWDGE dynamic
                # queues only fan out 5-wide). Every transfer is contiguous
                # in DRAM and per-partition contiguous in SBUF. lhsT streams
                # in per-slab pieces so the first matmul isn't gated on the
                # whole 1.5 MB. p-major row order within each slab:
                #   shard row = off*KP + p*L + j
                rhs = rpool.tile([KP, 2, L, EMB], MM_DT, tag="rhs")
                V3 = V_d[off * KP:(off + L) * KP, :].rearrange(
                    "(p j) e -> p j e", j=L)
                U3 = U_d[off * KP:(off + L) * KP, :].rearrange(
                    "(p j) e -> p j e", j=L)
                nc.gpsimd.dma_start(out=rhs[:, 0, :, :], in_=V3)
                nc.gpsimd.dma_start(out=rhs[:, 1, :, :], in_=U3)
                for j in range(L):
                    c = off + j
                    nc.tensor.matmul(
                        out=acc[:, :],
                        lhsT=lhsT_s[:, c, :],
                        rhs=rhs[:, :, j, :],
                        start=(c == 0),
                        stop=(c == NCHUNK - 1),
                    )
                off += L

            out_s = cpool.tile([M, 2 * EMB], F32)
            nc.vector.tensor_copy(out_s[:, :], acc[:, :])
            nc.sync.dma_start(out=out_d[:, :], in_=out_s[:, :])
    nc.compile()
    return nc


def make_in_maps(vo, vi, neg_samples, V, U):
    """Shard + relayout the full inputs into 8 per-core input maps.

    Host work is pure data movement: slicing, stacking and axis
    permutation. No arithmetic on values happens here.
    """
    vo = np.asarray(vo, dtype=np.float32)
    vi = np.asarray(vi, dtype=np.float32)
    neg = np.asarray(neg_samples, dtype=np.float32)
    V = np.asarray(V, dtype=np.float32)
    U = np.asarray(U, dtype=np.float32)

    in_maps = []
    for r in range(NCORES):
        lo, hi = r * SHARD, (r + 1) * SHARD
        # [12500, 31] = [vo | neg.T | vi] for this vocab shard
        W = np.concatenate([vo[lo:hi, None], neg[:, lo:hi].T, vi[lo:hi]],
                           axis=1)
        # p-major order within each slab: shard row = off*KP + p*L + j
        pieces = []
        off = 0
        for L in SLABS:
            seg = W[off * KP:(off + L) * KP].reshape(KP, L, M)
            pieces.append(seg)
            off += L
        lhsT_packed = np.ascontiguousarray(
            np.concatenate(pieces, axis=1)).reshape(KP, NCHUNK * M)
        in_maps.append({
            "lhsT_packed": lhsT_packed,
            "V_s": np.ascontiguousarray(V[lo:hi]),
            "U_s": np.ascontiguousarray(U[lo:hi]),
        })
    return in_maps


def combine_partials(partials):
    """Sum per-core partials and apply the scalar epilogue."""
    P = np.zeros((M, 2 * EMB), dtype=np.float64)
    for p in partials:
        P += p.astype(np.float64)
    vo_embed = P[0, :EMB]
    neg_embed = P[1:1 + KNEG, EMB:]
    vi_embed = P[1 + KNEG:, EMB:].sum(axis=0) / CTX

    def log_sigmoid(x):
        return -np.logaddexp(0.0, -x)

    left = log_sigmoid(vi_embed @ vo_embed)
    right = np.sum(log_sigmoid(-(neg_embed @ vi_embed)))
    return np.float32(-(left + right))


_NC = None


def kernel(vo, vi, neg_samples, V, U):
    global _NC
    if _NC is None:
        _NC = build_nc()
    in_maps = make_in_maps(vo, vi, neg_samples, V, U)
    res = run_bass_kernel_spmd(_NC, in_maps, list(range(NCORES)))
    return combine_partials([res.results[r]["partial"] for r in range(NCORES)])


# revision 28
# speedup vs baseline: 1.7102x; 1.6625x over previous
"""CBOW negative-sampling loss kernel for Trainium2 (8 NeuronCores, SPMD).

Reference computation (all fp32):
    vo_embed  = vo @ V                        # [128]
    vi_embed  = (U.T @ vi).mean(axis=1)       # [128]
    left      = log_sigmoid(vi_embed @ vo_embed)
    neg_embed = neg_samples @ U               # [20, 128]
    right     = sum(log_sigmoid(-(neg_embed @ vi_embed)))
    out       = -(left + right)

Strategy: shard the vocab dim (100000) across 8 cores (12500 rows each).
All the heavy work is 31 GEMVs sharing one contraction over vocab:
pack [vo | neg_0..neg_19 | vi_0..vi_9] (host, pure relayout) into a
31-column stationary operand; stream V|U row-chunks through the tensor
engine accumulating into PSUM.  Each core emits a [31, 256] partial; the
host sums partials over cores (the "psum"), averages the 10 vi rows, and
applies the scalar log-sigmoid epilogue.
"""

import numpy as np

import concourse.bacc as bacc
import concourse.bass as bass
import concourse.mybir as mybir
import concourse.tile as tile
from concourse.bass_utils import run_bass_kernel_spmd

# Problem shapes (hardcoded per spec nn_CBOW_55009941127479)
VOC = 100000
EMB = 128
CTX = 10
KNEG = 20
NCORES = 8
SHARD = VOC // NCORES          # 12500 true vocab rows per core
KP = 128                       # contraction rows per matmul chunk (SBUF partitions)
NCHUNK = 98                    # chunks per core (shard zero-padded to 128*98)
SHARD_PAD = KP * NCHUNK        # 12544: pad rows have zero weights -> no effect,
                               # but 128 partitions load all 16 SDMA engines
                               # evenly (125 left 3 engines at 2/3 load)
M = 1 + KNEG + CTX             # stationary columns: [vo, neg_0..19, vi_0..9]
# DMA slabs (chunks per slab). 8 chunks -> exactly 4096B per partition per
# transfer: SDMA engines pipeline 4KB packets at line rate (~26 GB/s each)
# but process larger descriptors at roughly half rate.
SLABS = [8] * 12 + [2]
USE_FP32R = True               # PE single-pass fp32r: 4x matmul throughput

# Vocab rows are processed in a p-major order within each slab so that every
# DMA is contiguous on both the DRAM and SBUF side:
#   shard row for (slab s, partition p, chunk-in-slab j) = s*KP*SLAB + p*SLAB + j
# The host packs lhsT in the same order, so all operands agree on the
# (equivalent, order-independent) contraction over vocab.

F32 = mybir.dt.float32
MM_DT = mybir.dt.float32r if USE_FP32R else F32


def build_nc():
    """Build the per-core Bass module (SPMD: same program on all 8 cores)."""
    nc = bacc.Bacc(
        "TRN2",
        target_bir_lowering=False,
        debug=False,
        num_devices=NCORES,
    )
    # DRAM inputs are declared float32r (bit-identical 4-byte layout; numpy
    # sees float32) so the DMAs are pure copies — the cast-during-DMA path
    # runs the SDMA engines at half rate.
    lhsT_d = nc.dram_tensor("lhsT_packed", [KP, NCHUNK * M], MM_DT,
                            kind="ExternalInput")
    V_d = nc.dram_tensor("V_s", [SHARD_PAD, EMB], MM_DT, kind="ExternalInput")
    U_d = nc.dram_tensor("U_s", [SHARD_PAD, EMB], MM_DT, kind="ExternalInput")
    out_d = nc.dram_tensor("partial", [M, 2 * EMB], F32, kind="ExternalOutput")

    with tile.TileContext(nc) as tc:
        with (
            tc.tile_pool(name="const", bufs=1) as cpool,
            tc.tile_pool(name="rhs", bufs=6) as rpool,
            tc.tile_pool(name="acc", bufs=1, space="PSUM") as ppool,
        ):
            lhsT_s = cpool.tile([KP, NCHUNK, M], MM_DT)
            lhsT3 = lhsT_d.rearrange("p (c m) -> p c m", m=M)

            # lhsT rides the ACT HWDGE ring, which is otherwise idle: its
            # bytes flow during the kernel entry window (before the SWDGE
            # Q7 library load finishes) instead of competing with the V/U
            # stream for the shared ~205 GB/s DMA budget. A small first
            # piece unblocks chunk 0's matmul early.
            nc.scalar.dma_start(out=lhsT_s[:, 0:16, :], in_=lhsT3[:, 0:16, :])
            nc.scalar.dma_start(out=lhsT_s[:, 16:, :], in_=lhsT3[:, 16:, :])

            # out[m, 0:128]   = w_m @ V_chunk   (used for m=0: vo)
            # out[m, 128:256] = w_m @ U_chunk   (used for m=1..30: neg, vi)
            acc = ppool.tile([M, 2 * EMB], F32)
            off = 0
            for L in SLABS:
                # All big transfers ride SWDGE (gpsimd), which sprays
                # descriptors across all 16 SDMA engines (the HWDGE dynamic
                # queues only fan out 5-wide). Every transfer is contiguous
                # in DRAM and per-partition contiguous in SBUF. lhsT streams
                # in per-slab pieces so the first matmul isn't gated on the
                # whole 1.5 MB. p-major row order within each slab:
                #   shard row = off*KP + p*L + j
                rhs = rpool.tile([KP, 2, L, EMB], MM_DT, tag="rhs")
                V3 = V_d[off * KP:(off + L) * KP, :].rearrange(
                    "(p j) e -> p j e", j=L)
                U3 = U_d[off * KP:(off + L) * KP, :].rearrange(
                    "(p j) e -> p j e", j=L)
                nc.gpsimd.dma_start(out=rhs[:, 0, :, :], in_=V3)
                nc.gpsimd.dma_start(out=rhs[:, 1, :, :], in_=U3)
                for j in range(L):
                    c = off + j
                    nc.tensor.matmul(
                        out=acc[:, :],
                        lhsT=lhsT_s[:, c, :],
                        rhs=rhs[:, :, j, :],
                        start=(c == 0),
                        stop=(c == NCHUNK - 1),
                    )
                off += L

            out_s = cpool.tile([M, 2 * EMB], F32)
            nc.vector.tensor_copy(out_s[:, :], acc[:, :])
            nc.sync.dma_start(out=out_d[:, :], in_=out_s[:, :])
    nc.compile()
    return nc


def make_in_maps(vo, vi, neg_samples, V, U):
    """Shard + relayout the full inputs into 8 per-core input maps.

    Host work is pure data movement: slicing, stacking and axis
    permutation. No arithmetic on values happens here.
    """
    vo = np.asarray(vo, dtype=np.float32)
    vi = np.asarray(vi, dtype=np.float32)
    neg = np.asarray(neg_samples, dtype=np.float32)
    V = np.asarray(V, dtype=np.float32)
    U = np.asarray(U, dtype=np.float32)

    in_maps = []
    for r in range(NCORES):
        lo, hi = r * SHARD, (r + 1) * SHARD
        # [12544, 31] = [vo | neg.T | vi] for this vocab shard, zero-padded
        # to 128*98 rows (pad rows have zero weight -> no contribution).
        W = np.zeros((SHARD_PAD, M), np.float32)
        W[:SHARD] = np.concatenate(
            [vo[lo:hi, None], neg[:, lo:hi].T, vi[lo:hi]], axis=1)
        # p-major order within each slab: padded row = off*KP + p*L + j
        pieces = []
        off = 0
        for L in SLABS:
            seg = W[off * KP:(off + L) * KP].reshape(KP, L, M)
            pieces.append(seg)
            off += L
        lhsT_packed = np.ascontiguousarray(
            np.concatenate(pieces, axis=1)).reshape(KP, NCHUNK * M)
        V_pad = np.zeros((SHARD_PAD, EMB), np.float32)
        V_pad[:SHARD] = V[lo:hi]
        U_pad = np.zeros((SHARD_PAD, EMB), np.float32)
        U_pad[:SHARD] = U[lo:hi]
        in_maps.append({
            "lhsT_packed": lhsT_packed,
            "V_s": V_pad,
            "U_s": U_pad,
        })
    return in_maps


def combine_partials(partials):
    """Sum per-core partials and apply the scalar epilogue."""
    P = np.zeros((M, 2 * EMB), dtype=np.float64)
    for p in partials:
        P += p.astype(np.float64)
    vo_embed = P[0, :EMB]
    neg_embed = P[1:1 + KNEG, EMB:]
    vi_embed = P[1 + KNEG:, EMB:].sum(axis=0) / CTX

    def log_sigmoid(x):
        return -np.logaddexp(0.0, -x)

    left = log_sigmoid(vi_embed @ vo_embed)
    right = np.sum(log_sigmoid(-(neg_embed @ vi_embed)))
    return np.float32(-(left + right))


_NC = None


def kernel(vo, vi, neg_samples, V, U):
    global _NC
    if _NC is None:
        _NC = build_nc()
    in_maps = make_in_maps(vo, vi, neg_samples, V, U)
    res = run_bass_kernel_spmd(_NC, in_maps, list(range(NCORES)))
    return combine_partials([res.results[r]["partial"] for r in range(NCORES)])
